# revision 8
# baseline (speedup 1.0000x reference)
"""DPLSTMCell kernel for 8 Trainium2 NeuronCores.

The reference module returns h_t[0] -- only batch row 0 of the LSTM cell
update -- so the full [B, 4H] gate GEMM is dead code.  The live computation
is two matvecs:

    gates[4H] = W_ih @ x0 + b_ih + W_hh @ h0 + b_hh      (x0 = x_t[0,0], h0 = h_prev[0,0])
    i,f,g,o   = split(gates, 4)
    c         = sigmoid(f) * c_prev[0] + sigmoid(i) * tanh(g)
    out[H]    = sigmoid(o) * tanh(c)

Sharding: split the H output dim across the 8 cores (128 h-indices each).
Core k needs rows {g*H + k*128 .. +128 | g in 0..3} of both weight matrices
(512 rows x 1024 each, 4 MB fp32 per core) -- no inter-core communication.

On-core mapping: the gate matvec runs on the TensorEngine with the input
vector as the (tiny) stationary operand:

    psum[1, 512] += v_chunk[128, 1].T @ Wt_chunk[128, 512]

over 16 contraction chunks (8 for W_ih, 8 for W_hh); the bias is folded in
as a 17th K=1 matmul against a constant-1 lhsT.  Weights are pre-transposed
on the host so each chunk DMA is contiguous.
"""

import numpy as np

import concourse.bass as bass
import concourse.mybir as mybir
from concourse import tile
from concourse.bass_utils import run_bass_kernel_spmd

B, D, H = 8192, 1024, 1024
NCORES = 8
HS = H // NCORES          # 128 output elements per core
R = 4 * HS                # 512 gate rows per core (i|f|g|o blocks)
KCH = (2 * D) // 128      # 16 contraction chunks (ih then hh)
AF = mybir.ActivationFunctionType
F32 = mybir.dt.float32

MM_DT = mybir.dt.float32  # matmul dtype (float32 / float32r / bfloat16)


def _np_dt(mm_dt):
    if mm_dt == mybir.dt.bfloat16:
        import ml_dtypes
        return np.dtype(ml_dtypes.bfloat16)
    return np.dtype(np.float32)


VB_W = KCH + 1 + R        # [128, 529]: cols 0:16 v-chunks, col 16 const-1, 17:529 bias row 0


def build_nc(mm_dt=MM_DT):
    nc = bass.Bass()
    w = nc.declare_dram_parameter("w", [KCH, 128, R], mm_dt, isOutput=False)
    vb = nc.declare_dram_parameter("vb", [128, VB_W], mm_dt, isOutput=False)
    c0 = nc.declare_dram_parameter("c0", [1, HS], F32, isOutput=False)
    out = nc.declare_dram_parameter("out", [1, HS], F32, isOutput=True)

    # Weight chunks grouped into 5 DMAs so the kernel issues <= 8 dma_starts
    # total (vb, c0, 5x w, out): each lands on its own HWDGE semaphore lane,
    # avoiding lane-reuse ordering waits -- walrus allows only ONE sync-wait
    # per instruction on this toolchain.
    W_GROUPS = [4, 3, 3, 3, 3]

    with tile.TileContext(nc) as tc:
        with (
            tc.tile_pool(name="wpool", bufs=len(W_GROUPS)) as wpool,
            tc.tile_pool(name="spool", bufs=1) as spool,
            tc.tile_pool(name="psum", bufs=1, space="PSUM") as psum,
        ):
            vb_sb = spool.tile([128, VB_W], mm_dt, tag="vb")
            c0_sb = spool.tile([1, HS], F32, tag="c0")
            nc.sync.dma_start(vb_sb[:], vb[:])
            nc.sync.dma_start(c0_sb[:], c0[:])
            # ACT-engine copy absorbs the c0-DMA wait; the f*c0 multiply below
            # then only needs the single ACT-chain wait (DVE ops reading a
            # recent DVE write would need a same-engine pipeline wait too).
            c0f = spool.tile([1, HS], F32, tag="c0f")
            nc.scalar.copy(c0f[:], c0_sb[:])

            gates = psum.tile([1, R], F32)

            # psum = 1.0 * (b_ih + b_hh) -- starts the accumulation group
            nc.tensor.matmul(
                gates[:], vb_sb[0:1, KCH:KCH + 1], vb_sb[0:1, KCH + 1:VB_W],
                start=True, stop=False,
            )
            j = 0
            for gi, gn in enumerate(W_GROUPS):
                wt = wpool.tile([128, gn, R], mm_dt, tag=f"w{gi}")
                src = w[j:j + gn].rearrange("n p r -> p n r")
                nc.sync.dma_start(wt[:], src)
                for t in range(gn):
                    nc.tensor.matmul(
                        gates[:], vb_sb[:, j:j + 1], wt[:, t, :],
                        start=False, stop=(j == KCH - 1),
                    )
                    j += 1

            acts = spool.tile([1, R], F32, tag="acts")   # [sig(i)|sig(f)|tanh(g)|sig(o)]
            nc.scalar.activation(acts[:, 0:2 * HS], gates[:, 0:2 * HS], AF.Sigmoid)
            nc.scalar.activation(acts[:, 2 * HS:3 * HS], gates[:, 2 * HS:3 * HS], AF.Tanh)
            nc.scalar.activation(acts[:, 3 * HS:4 * HS], gates[:, 3 * HS:4 * HS], AF.Sigmoid)

            ig = spool.tile([1, HS], F32, tag="ig")
            fc = spool.tile([1, HS], F32, tag="fc")
            ct = spool.tile([1, HS], F32, tag="ct")
            tct = spool.tile([1, HS], F32, tag="tct")
            ht = spool.tile([1, HS], F32, tag="ht")
            nc.vector.tensor_mul(ig[:], acts[:, 0:HS], acts[:, 2 * HS:3 * HS])
            nc.vector.tensor_mul(fc[:], acts[:, HS:2 * HS], c0f[:])
            nc.vector.tensor_add(ct[:], ig[:], fc[:])
            nc.scalar.activation(tct[:], ct[:], AF.Tanh)
            nc.vector.tensor_mul(ht[:], acts[:, 3 * HS:4 * HS], tct[:])
            nc.sync.dma_start(out[:], ht[:])

    # The Tile kernel-tail drain waits on every semaphore used (11 here), but
    # this walrus build has very few sync-wait slots per instruction.  In this
    # kernel every sem is transitively dominated by the out-DMA completion
    # (out dma <- ht <- tct <- ... <- all matmuls <- all input DMAs), so the
    # drain only needs the single wait on the out-DMA's update sem.
    last_dma = None
    big_drain = None
    for bb in nc.m.functions[0].blocks:
        for ins in bb.instructions:
            if type(ins).__name__ == "InstDMACopy":
                last_dma = ins
            if (type(ins).__name__ == "InstDrain"
                    and ins.sync_info and len(ins.sync_info.on_wait) > 1):
                assert big_drain is None
                big_drain = ins
    assert big_drain is not None and last_dma is not None
    out_sem = last_dma.sync_info.on_update[0].ant_name
    keep = [w for w in big_drain.sync_info.on_wait if w.ant_name == out_sem]
    assert len(keep) == 1
    big_drain.sync_info.on_wait = keep
    return nc


def prep_in_maps(x_t, h_prev, c_prev, weight_ih, weight_hh, bias_ih, bias_hh,
                 mm_dt=MM_DT):
    np_dt = _np_dt(mm_dt)
    x0 = np.asarray(x_t, dtype=np.float32)[0, 0]
    h0 = np.asarray(h_prev, dtype=np.float32)[0, 0]
    c0 = np.asarray(c_prev, dtype=np.float32)[0]
    wih = np.asarray(weight_ih, dtype=np.float32)
    whh = np.asarray(weight_hh, dtype=np.float32)
    bsum = (np.asarray(bias_ih, dtype=np.float32)
            + np.asarray(bias_hh, dtype=np.float32))

    v = np.concatenate([x0, h0]).reshape(KCH, 128).T          # col j = K-chunk j

    in_maps = []
    for k in range(NCORES):
        rows = (np.arange(4)[:, None] * H + k * HS + np.arange(HS)[None, :]).ravel()
        wk = np.concatenate([
            wih[rows].reshape(R, D // 128, 128).transpose(1, 2, 0),
            whh[rows].reshape(R, D // 128, 128).transpose(1, 2, 0),
        ], axis=0).astype(np_dt)                              # [16, 128, 512]
        vbk = np.zeros((128, VB_W), np.float32)
        vbk[:, :KCH] = v
        vbk[0, KCH] = 1.0
        vbk[0, KCH + 1:] = bsum[rows]
        in_maps.append({
            "w": np.ascontiguousarray(wk),
            "vb": vbk.astype(np_dt),
            "c0": np.ascontiguousarray(c0[k * HS:(k + 1) * HS].reshape(1, HS)),
        })
    return in_maps


_NC_CACHE = {}


def run(inputs, mm_dt=MM_DT, trace=False, **spmd_kwargs):
    if mm_dt not in _NC_CACHE:
        _NC_CACHE[mm_dt] = build_nc(mm_dt)
    nc = _NC_CACHE[mm_dt]
    in_maps = prep_in_maps(**inputs, mm_dt=mm_dt)
    res = run_bass_kernel_spmd(
        nc, in_maps, core_ids=list(range(NCORES)), trace=trace, **spmd_kwargs
    )
    out = np.concatenate(
        [np.asarray(res.results[k]["out"]).reshape(HS) for k in range(NCORES)]
    ).astype(np.float32)
    return out, res


def kernel(**inputs):
    out, _ = run(inputs)
    return out
